# revision 13
# baseline (speedup 1.0000x reference)
"""FastSelfAttention Trainium2 kernel (batched two-phase, bf16 I/O).

Reference computation (B=4, S=4096, D=1024):
    h  = layer_norm(hidden_states, g, b)
    q  = h @ Wq.T ; k = h @ Wk.T ; v = q
    qw = exp((q @ wq_att) / sqrt(D) + mask)
    pq = cumsum(qw * q, S) / cumsum(qw, S)
    mk = pq * k
    kw = exp((mk @ wk_att) / sqrt(D) + mask)
    pk = cumsum(kw * mk, S) / cumsum(kw, S)
    out = pk * v

Sharding: 8 cores = 4 batches x 2 halves of the feature (e) dimension.
Layout on device is feature-major [e, s]; cumsum runs along the free
(s) axis via DVE tensor_tensor_scan, chained across s-chunks with
carry columns.  The second pooling's logit l2 needs the full e range:
sweep-1 runs over a 4-chunk super-batch, one pairwise AllReduce
([[0,1],[2,3],[4,5],[6,7]]) combines the halves, then pool-2 replays
those chunks while the next super-batch's sweep-1 overlaps the AR.

LayerNorm folding: with xs[d,s] = h[d,s]*rstd[s] the projection is
    q[e,s] = sum_d W'q[e,d] xs[d,s] + nmur[s] colsq[e] + cq[e]
(nmur = -mu*rstd); the two rank-1 corrections ride one K=2 matmul
(stationary [colsq; cq], moving rows [nmur; ones]).
l1 = rstd * ((vqp - colsvq/D).h) + (b.vq/sqrt(D) + mask) so the mean
subtraction is folded into the l1 stationary host-side.

Everything streams in bf16 (h input, weights, q/k/mk, output); scans
and denominators accumulate in f32.
"""

import numpy as np
import ml_dtypes

import concourse.bass as bass
import concourse.bacc as bacc
import concourse.mybir as mybir
import concourse.tile as tile
from concourse.bass_utils import run_bass_kernel_spmd

dt = mybir.dt
AF = mybir.ActivationFunctionType
OP = mybir.AluOpType

B, S, D = 4, 4096, 1024
EH = D // 2          # e-half per core
NC = 8               # cores
SC = 512             # s-chunk
NSC = S // SC        # 8 s-chunks
NB = 2               # AllReduce super-batches
CPB = NSC // NB      # chunks per super-batch (4)
SB = SC * CPB        # tokens per super-batch (2048)
ND = D // 128        # 8 d-chunks
NE = EH // 128       # 4 e-chunks per core
INV_SQRT_D = 1.0 / np.sqrt(np.float32(D))
EPS = 1e-5

_prog_cache = {}


def _build_program(no_collective=False):
    key = ("ncb", no_collective)
    if key in _prog_cache:
        return _prog_cache[key]

    nc = bacc.Bacc("TRN2", num_devices=NC)
    f32, bf16 = dt.float32, dt.bfloat16

    # ---- external I/O (all big tensors bf16) ----
    hb = nc.dram_tensor("hb", [D, S], bf16, kind="ExternalInput")
    wqT = nc.dram_tensor("wqT", [D, EH], bf16, kind="ExternalInput")
    wkT = nc.dram_tensor("wkT", [D, EH], bf16, kind="ExternalInput")
    svq_in = nc.dram_tensor("svq", [ND, 128], bf16, kind="ExternalInput")
    ccq_in = nc.dram_tensor("ccq", [2, EH], bf16, kind="ExternalInput")
    cck_in = nc.dram_tensor("cck", [2, EH], bf16, kind="ExternalInput")
    wkp_in = nc.dram_tensor("wkp", [NE, 128], bf16, kind="ExternalInput")
    mrow1_in = nc.dram_tensor("mrow1", [1, S], f32, kind="ExternalInput")
    mrow2_in = nc.dram_tensor("mrow2", [1, S], f32, kind="ExternalInput")
    r2init_in = nc.dram_tensor("r2init", [2, SC], bf16, kind="ExternalInput")

    outT = nc.dram_tensor("outT", [EH, S], bf16, kind="ExternalOutput")

    with tile.TileContext(nc) as tc:
        with (
            tc.tile_pool(name="const", bufs=1) as cpool,
            tc.tile_pool(name="persist", bufs=1) as ppool,
            tc.tile_pool(name="rows", bufs=1) as rows,
            tc.tile_pool(name="work", bufs=2) as wk,
            tc.tile_pool(name="work1", bufs=1) as wk1,
            tc.tile_pool(name="psA", bufs=2, space="PSUM") as psA,
            tc.tile_pool(name="psB", bufs=2, space="PSUM") as psB,
            tc.tile_pool(name="psR", bufs=1, space="PSUM") as psR,
            tc.tile_pool(name="psL2", bufs=1, space="PSUM") as psL2,
            tc.tile_pool(name="dram", bufs=1, space="DRAM") as dpool,
        ):
            # ---- resident constants (one DMA each) ----
            wq_t = cpool.tile([128, ND, EH], bf16, tag="wq")
            wk_t = cpool.tile([128, ND, EH], bf16, tag="wk")
            nc.sync.dma_start(
                out=wq_t[:], in_=wqT.rearrange("(a p) e -> p a e", p=128))
            nc.sync.dma_start(
                out=wk_t[:], in_=wkT.rearrange("(a p) e -> p a e", p=128))

            svq_t = cpool.tile([128, ND], bf16, tag="svq")
            nc.sync.dma_start(out=svq_t[:], in_=svq_in.transpose([1, 0]))

            ccq_t = cpool.tile([2, EH], bf16, tag="ccq")
            cck_t = cpool.tile([2, EH], bf16, tag="cck")
            nc.sync.dma_start(out=ccq_t[:], in_=ccq_in[:])
            nc.sync.dma_start(out=cck_t[:], in_=cck_in[:])

            wkp_t = cpool.tile([128, NE], bf16, tag="wkp")
            nc.sync.dma_start(out=wkp_t[:], in_=wkp_in.transpose([1, 0]))

            r2 = cpool.tile([2, SC], bf16, tag="r2")
            nc.sync.dma_start(out=r2[:], in_=r2init_in[:])

            ones_rk1 = cpool.tile([1, 128], bf16, tag="ones_rk1")
            nc.vector.memset(ones_rk1[:], 1.0)
            ones1 = cpool.tile([128, 1], bf16, tag="ones1")
            nc.vector.memset(ones1[:], 1.0)
            eps_t = cpool.tile([1, 1], f32, tag="eps")
            nc.vector.memset(eps_t[:], EPS)

            # ---- persistent state ----
            carry_q = ppool.tile([128, NE], f32, tag="carry_q")
            carry_k = ppool.tile([128, NE], f32, tag="carry_k")
            carry_d = ppool.tile([1, 2], f32, tag="carry_d")
            nc.vector.memset(carry_q[:], 0.0)
            nc.vector.memset(carry_k[:], 0.0)
            nc.vector.memset(carry_d[:], 0.0)

            l2p_dram = dpool.tile([1, S], f32, tag="l2p")
            l2f_dram = dpool.tile([1, S], f32, tag="l2f")

            for g in range(NB):
                g0 = g * SB
                # super-batch state (double-buffered across batches)
                q_t = {}
                mk_t = {}
                l2acc = rows.tile([1, SB], f32, tag="l2acc")

                # ================= sweep 1 =================
                for cc in range(CPB):
                    s0 = g0 + cc * SC

                    ht_c = wk.tile([128, ND, SC], bf16, tag="ht")
                    nc.sync.dma_start(
                        out=ht_c[:],
                        in_=hb.rearrange("(a p) s -> p a s",
                                         p=128)[:, :, s0:s0 + SC])

                    sq_t = wk1.tile([128, ND, SC], bf16, tag="sqxs")
                    nc.scalar.activation(sq_t[:], ht_c[:], AF.Square)

                    # stats + l1 rows
                    st_ps = psR.tile([1, SC], f32, tag="st")
                    for d in range(ND):
                        nc.tensor.matmul(st_ps[:], ones1[:], ht_c[:, d, :],
                                         start=(d == 0), stop=(d == ND - 1))
                    l1_ps = psR.tile([1, SC], f32, tag="l1")
                    for d in range(ND):
                        nc.tensor.matmul(l1_ps[:], svq_t[:, d:d + 1],
                                         ht_c[:, d, :],
                                         start=(d == 0), stop=(d == ND - 1))
                    sxx_ps = psR.tile([1, SC], f32, tag="sxx")
                    for d in range(ND):
                        nc.tensor.matmul(sxx_ps[:], ones1[:], sq_t[:, d, :],
                                         start=(d == 0), stop=(d == ND - 1))

                    # LN rows
                    negmu = rows.tile([1, SC], f32, tag="negmu")
                    nc.vector.tensor_scalar_mul(negmu[:], st_ps[:], -1.0 / D)
                    musq = rows.tile([1, SC], f32, tag="scratch")
                    nc.scalar.activation(musq[:], st_ps[:], AF.Square,
                                         scale=1.0 / D)
                    var = rows.tile([1, SC], f32, tag="var")
                    nc.vector.scalar_tensor_tensor(
                        var[:], sxx_ps[:], 1.0 / D, musq[:],
                        OP.mult, OP.subtract)
                    sd = rows.tile([1, SC], f32, tag="scratch")
                    nc.scalar.activation(sd[:], var[:], AF.Sqrt, bias=eps_t[:])
                    rstd = rows.tile([1, SC], f32, tag="rstd")
                    rscr = rows.tile([1, SC], f32, tag="rscr")
                    nc.vector.reciprocal_approx_accurate(rstd[:], sd[:],
                                                         rscr[:])
                    rstd_h = rows.tile([1, SC], bf16, tag="rstd_h")
                    nc.vector.tensor_copy(rstd_h[:], rstd[:])

                    # rank-1 moving rows [nmur; ones] (row1 DMA-initialized)
                    nc.vector.tensor_mul(r2[0:1, :], negmu[:], rstd[:])

                    # rstd broadcast -> xs
                    rb_ps = psB.tile([128, SC], f32, tag="bcast")
                    nc.tensor.matmul(rb_ps[:], ones_rk1[:], rstd_h[:],
                                     start=True, stop=True)
                    xs_t = wk1.tile([128, ND, SC], bf16, tag="sqxs")
                    for d in range(ND):
                        nc.vector.tensor_mul(xs_t[:, d, :], ht_c[:, d, :],
                                             rb_ps[:])

                    # l1 -> qw
                    l1f = rows.tile([1, SC], f32, tag="l1f")
                    nc.vector.tensor_mul(l1f[:], l1_ps[:], rstd[:])
                    m1s = rows.tile([1, SC], f32, tag="m1s")
                    nc.sync.dma_start(out=m1s[:], in_=mrow1_in[:, s0:s0 + SC])
                    l1b = rows.tile([1, SC], f32, tag="l1b")
                    nc.vector.tensor_add(l1b[:], l1f[:], m1s[:])
                    qw = rows.tile([1, SC], bf16, tag="qw")
                    nc.scalar.activation(qw[:], l1b[:], AF.Exp)

                    qb_ps = psB.tile([128, SC], f32, tag="bcast")
                    nc.tensor.matmul(qb_ps[:], ones_rk1[:], qw[:],
                                     start=True, stop=True)

                    # den1 scan + reciprocal
                    den1 = rows.tile([1, SC], f32, tag="den1")
                    init1 = 0.0 if s0 == 0 else carry_d[:, 0:1]
                    nc.vector.tensor_tensor_scan(
                        den1[:], qw[:], qw[:], init1, OP.add, OP.bypass)
                    nc.vector.tensor_copy(carry_d[:, 0:1], den1[:, SC - 1:SC])
                    rden1 = rows.tile([1, SC], f32, tag="rden1")
                    nc.vector.reciprocal_approx_accurate(
                        rden1[:], den1[:], rscr[:])
                    rden1h = rows.tile([1, SC], bf16, tag="rden1h")
                    nc.vector.tensor_copy(rden1h[:], rden1[:])

                    # phase A: projections + pool1 scans
                    n1_t = [None] * NE
                    for e in range(NE):
                        es = slice(e * 128, (e + 1) * 128)
                        q_ps = psA.tile([128, SC], f32, tag="proj")
                        for d in range(ND):
                            nc.tensor.matmul(
                                q_ps[:], wq_t[:, d, es], xs_t[:, d, :],
                                start=(d == 0), stop=False)
                        nc.tensor.matmul(q_ps[:], ccq_t[:, es], r2[:],
                                         start=False, stop=True)
                        qt = wk.tile([128, SC], bf16, tag=f"q{e}_{cc}")
                        nc.scalar.copy(qt[:], q_ps[:])
                        q_t[(e, cc)] = qt

                        k_ps = psA.tile([128, SC], f32, tag="proj")
                        for d in range(ND):
                            nc.tensor.matmul(
                                k_ps[:], wk_t[:, d, es], xs_t[:, d, :],
                                start=(d == 0), stop=False)
                        nc.tensor.matmul(k_ps[:], cck_t[:, es], r2[:],
                                         start=False, stop=True)
                        kt = wk1.tile([128, SC], bf16, tag=f"k{e}")
                        nc.scalar.copy(kt[:], k_ps[:])

                        u1 = wk1.tile([128, SC], bf16, tag="u1")
                        nc.vector.tensor_mul(u1[:], qb_ps[:], qt[:])
                        n1 = wk1.tile([128, SC], f32, tag=f"n1{e}")
                        initq = 0.0 if s0 == 0 else carry_q[:, e:e + 1]
                        nc.vector.tensor_tensor_scan(
                            n1[:], u1[:], u1[:], initq, OP.add, OP.bypass)
                        nc.vector.tensor_copy(carry_q[:, e:e + 1],
                                              n1[:, SC - 1:SC])
                        n1_t[e] = (n1, kt)

                    db_ps = psB.tile([128, SC], f32, tag="bcast")
                    nc.tensor.matmul(db_ps[:], ones_rk1[:], rden1h[:],
                                     start=True, stop=True)

                    # phase B: mk + l2 partial
                    l2_ps = psL2.tile([1, SC], f32, tag="l2")
                    for e in range(NE):
                        n1, kt = n1_t[e]
                        pq = wk1.tile([128, SC], bf16, tag="pq")
                        nc.vector.tensor_mul(pq[:], n1[:], db_ps[:])
                        mk = wk.tile([128, SC], bf16, tag=f"mk{e}_{cc}")
                        nc.vector.tensor_mul(mk[:], pq[:], kt[:])
                        mk_t[(e, cc)] = mk
                        nc.tensor.matmul(l2_ps[:], wkp_t[:, e:e + 1], mk[:],
                                         start=(e == 0), stop=(e == NE - 1))
                    nc.vector.tensor_copy(
                        l2acc[:, cc * SC:(cc + 1) * SC], l2_ps[:])

                # ============ AllReduce (one per super-batch) ============
                nc.sync.dma_start(out=l2p_dram[:, g0:g0 + SB], in_=l2acc[:])
                if no_collective:
                    nc.sync.dma_start(out=l2f_dram[:, g0:g0 + SB],
                                      in_=l2p_dram[:, g0:g0 + SB])
                else:
                    nc.gpsimd.collective_compute(
                        "AllReduce", OP.add,
                        replica_groups=[[0, 1], [2, 3], [4, 5], [6, 7]],
                        ins=[l2p_dram[:, g0:g0 + SB]],
                        outs=[l2f_dram[:, g0:g0 + SB]],
                    )
                l2s = rows.tile([1, SB], f32, tag="l2s")
                nc.sync.dma_start(out=l2s[:], in_=l2f_dram[:, g0:g0 + SB])
                m2s = rows.tile([1, SB], f32, tag="m2sg")
                nc.sync.dma_start(out=m2s[:], in_=mrow2_in[:, g0:g0 + SB])

                # ================= pool 2 =================
                for cc in range(CPB):
                    s0 = g0 + cc * SC
                    sl = slice(cc * SC, (cc + 1) * SC)
                    lg2 = rows.tile([1, SC], f32, tag="lg2")
                    nc.vector.tensor_add(lg2[:], l2s[:, sl], m2s[:, sl])
                    kw = rows.tile([1, SC], bf16, tag="kw")
                    nc.scalar.activation(kw[:], lg2[:], AF.Exp)
                    kb_ps = psB.tile([128, SC], f32, tag="bcast")
                    nc.tensor.matmul(kb_ps[:], ones_rk1[:], kw[:],
                                     start=True, stop=True)

                    den2 = rows.tile([1, SC], f32, tag="den2")
                    init2 = 0.0 if s0 == 0 else carry_d[:, 1:2]
                    nc.vector.tensor_tensor_scan(
                        den2[:], kw[:], kw[:], init2, OP.add, OP.bypass)
                    nc.vector.tensor_copy(carry_d[:, 1:2], den2[:, SC - 1:SC])
                    rden2 = rows.tile([1, SC], f32, tag="rden2")
                    rscr2 = rows.tile([1, SC], f32, tag="rscr2")
                    nc.vector.reciprocal_approx_accurate(
                        rden2[:], den2[:], rscr2[:])
                    rden2h = rows.tile([1, SC], bf16, tag="rden2h")
                    nc.vector.tensor_copy(rden2h[:], rden2[:])
                    d2_ps = psB.tile([128, SC], f32, tag="bcast")
                    nc.tensor.matmul(d2_ps[:], ones_rk1[:], rden2h[:],
                                     start=True, stop=True)

                    o_t = wk1.tile([128, NE, SC], bf16, tag="o")
                    for e in range(NE):
                        u2 = wk1.tile([128, SC], bf16, tag="u2")
                        nc.vector.tensor_mul(u2[:], kb_ps[:],
                                             mk_t[(e, cc)][:])
                        n2 = wk1.tile([128, SC], f32, tag="n2")
                        initk = 0.0 if s0 == 0 else carry_k[:, e:e + 1]
                        nc.vector.tensor_tensor_scan(
                            n2[:], u2[:], u2[:], initk, OP.add, OP.bypass)
                        nc.vector.tensor_copy(carry_k[:, e:e + 1],
                                              n2[:, SC - 1:SC])
                        pk = wk1.tile([128, SC], bf16, tag="pk")
                        nc.vector.tensor_mul(pk[:], n2[:], d2_ps[:])
                        nc.vector.tensor_mul(o_t[:, e, :], pk[:],
                                             q_t[(e, cc)][:])
                    nc.sync.dma_start(
                        out=outT.rearrange("(a p) s -> p a s",
                                           p=128)[:, :, s0:s0 + SC],
                        in_=o_t[:])

    nc.finalize()
    _prog_cache[key] = nc
    return nc


def _host_prep(hidden_states, attention_mask, Wq, wq_att, Wk, wk_att, ln_g, ln_b):
    """Build the 8 per-core input maps."""
    f4 = np.float32
    g = np.asarray(ln_g, f4)
    bb = np.asarray(ln_b, f4)
    Wq = np.asarray(Wq, f4)
    Wk = np.asarray(Wk, f4)
    wq_att = np.asarray(wq_att, f4)[:, 0]
    wk_att = np.asarray(wk_att, f4)[:, 0]
    h = np.asarray(hidden_states, f4)
    am = np.asarray(attention_mask, f4)

    Wqp = Wq * g[None, :]           # [e,d]
    Wkp = Wk * g[None, :]
    wqT_full = np.ascontiguousarray(Wqp.T)   # [d,e]
    wkT_full = np.ascontiguousarray(Wkp.T)
    cq_full = Wq @ bb               # [e]
    ck_full = Wk @ bb
    colsq_full = Wqp.sum(axis=1)    # [e]
    colsk_full = Wkp.sum(axis=1)

    vq = Wq.T @ wq_att              # [d]
    vqp = (g * vq) * INV_SQRT_D     # [d]
    cvq = float(bb @ vq) * INV_SQRT_D
    colsvq = float(vqp.sum())
    wkp_full = (wk_att * INV_SQRT_D).astype(f4)

    maskb = (1.0 - am) * -10000.0   # [B,S]

    def bf(a):
        return np.ascontiguousarray(np.asarray(a, f4).astype(ml_dtypes.bfloat16))

    # l1 stationary with mean-subtraction folded: vqp - colsvq/D
    svq = (vqp - colsvq / D).reshape(ND, 128)

    in_maps = []
    for core in range(NC):
        b, half = divmod(core, 2)
        sl = slice(half * EH, (half + 1) * EH)
        ccq = np.stack([colsq_full[sl], cq_full[sl]], axis=0)   # [2, EH]
        cck = np.stack([colsk_full[sl], ck_full[sl]], axis=0)
        in_maps.append({
            "hb": bf(h[b].T),
            "wqT": bf(wqT_full[:, sl]),
            "wkT": bf(wkT_full[:, sl]),
            "svq": bf(svq),
            "ccq": bf(ccq),
            "cck": bf(cck),
            "wkp": bf(wkp_full[sl].reshape(NE, 128)),
            "mrow1": np.ascontiguousarray((maskb[b] + cvq).reshape(1, S)),
            "mrow2": np.ascontiguousarray(maskb[b].reshape(1, S)),
            "r2init": bf(np.vstack([np.zeros(SC, f4), np.ones(SC, f4)])),
        })
    return in_maps


def kernel(**inputs):
    import time as _time
    nc = _build_program()
    in_maps = _host_prep(**inputs)
    res = None
    last = None
    for _attempt in range(3):
        try:
            res = run_bass_kernel_spmd(nc, in_maps, core_ids=list(range(NC)))
            break
        except Exception as e:  # transient first-exec device faults self-heal
            last = e
            _time.sleep(3)
    if res is None:
        raise last
    out = np.empty((B, S, D), np.float32)
    for core in range(NC):
        b, half = divmod(core, 2)
        out[b, :, half * EH:(half + 1) * EH] = \
            res.results[core]["outT"].astype(np.float32).T
    return out
